# revision 25
# baseline (speedup 1.0000x reference)
"""Bass/Tile kernel for masked dot-product attention on 8 Trainium2 cores.

Problem: queries/keys/values [128, 1024, 64] fp32, valid_lens [128] int32.
  out[b] = softmax(mask(Q K^T / 8, valid_lens[b])) @ V

Strategy:
  * Shard the 128 batch*heads across 8 cores, 16 head-slots per core.
    Heads are sorted by valid_len (descending) and dealt round-robin so
    every core gets the same per-slot chunk count -> one SPMD program.
  * Per head, only ceil(valid_len/128) key chunks contribute (the rest are
    fully masked -> softmax weight exactly 0), so the program is
    specialized to skip them (~45% of the work for uniform valid_lens).
  * Layout: compute S^T = K Q^T chunkwise on the PE ([128 k x 1024 q]),
    so the PV matmul can consume P^T directly as the moving operand.
    Masking + 1/sqrt(d) scaling + exp run as a single ScalarE activation
    (bias = per-partition mask column of 0 / -1e6; no max subtraction is
    needed: scores are bounded and exp(-1e6) underflows to exactly 0,
    matching the fp32 reference).
  * Softmax denominators come free: a ones-column is appended to V, so
    the PV accumulation produces [O^T ; sum_k P^T] in one pass.
    Normalization happens after a final PE transpose, where the
    denominator is a per-partition scalar.
  * Heads with valid_len == 0 (reference: uniform attention) are fixed up
    on the host with the exact reference semantics (mean of V).
"""

import math
from contextlib import ExitStack

import numpy as np

import concourse.bass as bass  # noqa: F401  (engine namespaces live on the nc)
import concourse.mybir as mybir
import concourse.tile as tile
from concourse import bacc
from concourse.bass_utils import run_bass_kernel_spmd
from concourse.masks import make_identity

BH, L, D = 128, 1024, 64
NCORES = 8
SLOTS = BH // NCORES  # 16
CHUNK = 128
NCH = L // CHUNK  # 8
MASK_VALUE = -1000000.0
F32 = mybir.dt.float32
MM_DT = mybir.dt.float16  # 1 cyc/row on PE, ~2^-11 operand quantization

_program_cache: dict = {}


def _build_program(m_list):
    nc = bacc.Bacc("TRN2", target_bir_lowering=False, debug=False)
    q_d = nc.dram_tensor("q", [SLOTS, L, D], F32, kind="ExternalInput").ap()
    k_d = nc.dram_tensor("k", [SLOTS, L, D], F32, kind="ExternalInput").ap()
    v_d = nc.dram_tensor("v", [SLOTS, L, D], F32, kind="ExternalInput").ap()
    mb_d = nc.dram_tensor("mb", [CHUNK, SLOTS * NCH], F32, kind="ExternalInput").ap()
    scr = [
        nc.dram_tensor(f"scr{j}", [2 * L, 2 * D], MM_DT).ap() for j in range(SLOTS)
    ]
    o_d = nc.dram_tensor("o", [SLOTS, L, D], F32, kind="ExternalOutput").ap()

    Exp = mybir.ActivationFunctionType.Exp

    with tile.TileContext(nc) as tc, ExitStack() as ctx:
        const = ctx.enter_context(tc.tile_pool(name="const", bufs=1))
        ident = const.tile([128, 128], F32)
        make_identity(nc, ident)
        mb = const.tile([CHUNK, SLOTS * NCH], F32)
        nc.sync.dma_start(mb[:], mb_d[:])
        ones = const.tile([128, 1], F32)
        nc.gpsimd.memset(ones[:], 1.0)

        qpf_p = ctx.enter_context(tc.tile_pool(name="qpf", bufs=4))
        qpb_p = ctx.enter_context(tc.tile_pool(name="qpb", bufs=3))
        qt_p = ctx.enter_context(tc.tile_pool(name="qt", bufs=4))
        kt_p = ctx.enter_context(tc.tile_pool(name="kt", bufs=3))
        vnat_p = ctx.enter_context(tc.tile_pool(name="vnat", bufs=4))
        vp_p = ctx.enter_context(tc.tile_pool(name="vp", bufs=4))
        pt_p = ctx.enter_context(tc.tile_pool(name="pt", bufs=4))
        ot_p = ctx.enter_context(tc.tile_pool(name="ot", bufs=2))
        osb_p = ctx.enter_context(tc.tile_pool(name="osb", bufs=4))
        rec_p = ctx.enter_context(tc.tile_pool(name="rec", bufs=4))

        # PSUM: 8 banks. "s": S^T tiles + epilogue transposes (2 x 2 banks);
        # "ops": PV accumulators (2 x 2 banks).
        s_ps = ctx.enter_context(tc.tile_pool(name="s", bufs=3, space="PSUM"))
        o_ps = ctx.enter_context(tc.tile_pool(name="ops", bufs=2, space="PSUM"))

        # Dense matmul burst to flip the PE HAM clock-gate to full rate
        # (~3.4us of contiguous activity required) before real work starts.
        warm = const.tile([128, 512], MM_DT, tag="warm")
        nc.gpsimd.memset(warm[:], 0.5)
        wps = o_ps.tile([128, 512], F32, tag="ops")  # noqa
        for i in range(14):
            nc.tensor.matmul(
                wps[:], warm[:, 0:128], warm[:], start=True, stop=True
            )

        zt = const.tile([128, L], MM_DT, tag="zt")
        nc.gpsimd.memset(zt[:], 0.0)

        def load_head(j, m):
            """Q and K panels -> one fp16 DRAM panel -> XBAR-transposed SBUF.

            Returns qkt [128, L + m*CHUNK] fp16: cols 0:L are Q^T (rows 0-63
            real, 64-127 zero), cols L: are K^T chunks.
            """
            nrows = L + m * CHUNK
            nch = NCH + m
            pf = qpf_p.tile([128, L], F32, tag="pf", name=f"pf{j}")
            nc.sync.dma_start(
                pf[:, 0 : NCH * D].rearrange("p (c d) -> p c d", d=D),
                q_d[j].rearrange("(c p) d -> p c d", p=CHUNK),
            )
            nc.sync.dma_start(
                pf[:, NCH * D : NCH * D + m * D].rearrange("p (c d) -> p c d", d=D),
                k_d[j, 0 : m * CHUNK].rearrange("(c p) d -> p c d", p=CHUNK),
            )
            # Staging: data in cols c*128+0:64, zeros in c*128+64:128, so one
            # contiguous store fills the whole panel (incl. the zero half).
            pb = qpb_p.tile([128, 2 * L], MM_DT, tag="pb", name=f"pb{j}")
            pb3 = pb[:, 0 : nch * CHUNK].rearrange("p (c d) -> p c d", d=CHUNK)
            nc.gpsimd.memset(pb3[:, :, D : CHUNK], 0.0)
            nc.vector.tensor_copy(
                pb3[:, :, 0:D],
                pf[:, 0 : nch * D].rearrange("p (c d) -> p c d", d=D),
            )
            nc.sync.dma_start(
                scr[j][0:nrows].rearrange("(c p) d -> p c d", p=CHUNK),
                pb3,
            )
            qkt = qt_p.tile([128, 2 * L], MM_DT, tag="qt", name=f"qkt{j}")
            nc.sync.dma_start_transpose(qkt[:, 0:nrows], scr[j][0:nrows, :])

            # V chunks: [V_c | ones | zeros] -> M=128 stationary panels.
            vnat = vnat_p.tile([128, NCH * D], F32, tag="vnat", name=f"vn{j}")
            nc.gpsimd.dma_start(
                vnat[:, 0 : m * D].rearrange("p (c d) -> p c d", d=D),
                v_d[j, 0 : m * CHUNK].rearrange("(c p) d -> p c d", p=CHUNK),
            )
            vp = vp_p.tile([128, NCH * CHUNK], MM_DT, tag="vp", name=f"vp{j}")
            nc.vector.tensor_copy(
                vp[:].rearrange("p (c e) -> p c e", e=CHUNK)[:, 0:m, 0:D],
                vnat[:, 0 : m * D].rearrange("p (c d) -> p c d", d=D),
            )
            for c in range(m):
                base = c * CHUNK
                nc.vector.tensor_copy(vp[:, base + D : base + D + 1], ones[:])
                nc.vector.tensor_copy(
                    vp[:, base + D + 1 : base + CHUNK], zt[:, 0 : CHUNK - D - 1]
                )
            return qkt, vp

        epilogue_pending = []
        pending = [load_head(jj, m_list[jj]) for jj in range(min(3, SLOTS))]
        for j in range(SLOTS):
            m = m_list[j]
            qkt, vp = pending.pop(0)
            if j + 3 < SLOTS:
                pending.append(load_head(j + 3, m_list[j + 3]))

            opsum = [
                o_ps.tile([128, 512], F32, tag="ops", name=f"op{j}_{h}")
                for h in range(2)
            ]
            pts = {}

            def emit_pv(c):
                vl = vp[:, c * CHUNK : (c + 1) * CHUNK]
                for h in range(2):
                    nc.tensor.matmul(
                        opsum[h][:],
                        vl,
                        pts[c][:, h * 512 : (h + 1) * 512],
                        start=(c == 0),
                        stop=(c == m - 1),
                    )

            for c in range(m):
                s = s_ps.tile([128, L], F32, tag="s", name=f"s_{j}_{c}")
                for h in range(2):
                    nc.tensor.matmul(
                        s[:, h * 512 : (h + 1) * 512],
                        qkt[:, L + c * 128 : L + (c + 1) * 128],
                        qkt[:, h * 512 : (h + 1) * 512],
                        start=True,
                        stop=True,
                    )
                if c >= 1:
                    emit_pv(c - 1)
                if c == 1 and epilogue_pending:
                    epilogue_pending.pop(0)()
                pts[c] = pt_p.tile([128, L], MM_DT, tag="pt", name=f"pt{j}_{c}")
                col = j * NCH + c
                nc.scalar.activation(
                    pts[c][:], s[:], Exp, bias=mb[:, col : col + 1], scale=0.125
                )
            emit_pv(m - 1)

            def make_epilogue(j, m, opsum):
                def epi():
                    # Transpose [O^T ; denom] back (4 blocks per PSUM bank),
                    # normalize, one store.
                    ot = ot_p.tile([65, L], F32, tag="ot", name=f"ot{j}")
                    for h in range(2):
                        nc.vector.tensor_copy(
                            ot[:, h * 512 : (h + 1) * 512], opsum[h][0:65, :]
                        )
                    osb = osb_p.tile([128, NCH * D], F32, tag="osb", name=f"osb{j}")
                    for gg in range(2):
                        tt = s_ps.tile(
                            [128, 4 * 65], F32, tag="s", name=f"tt{j}_{gg}"
                        )
                        for g4 in range(4):
                            g = 4 * gg + g4
                            nc.tensor.transpose(
                                tt[:, g4 * 65 : g4 * 65 + 65],
                                ot[:, g * 128 : (g + 1) * 128],
                                ident[0:65, 0:65],
                            )
                        rec = rec_p.tile(
                            [128, 4], F32, tag="rec", name=f"rec{j}_{gg}"
                        )
                        nc.vector.reciprocal(
                            rec[:],
                            tt[:].rearrange("p (g e) -> p g e", e=65)[:, :, 64],
                        )
                        for g4 in range(4):
                            g = 4 * gg + g4
                            nc.vector.tensor_scalar_mul(
                                osb[:, g * D : (g + 1) * D],
                                tt[:, g4 * 65 : g4 * 65 + 64],
                                rec[:, g4 : g4 + 1],
                            )
                    nc.gpsimd.dma_start(
                        o_d[j].rearrange("(g p) d -> p g d", p=CHUNK),
                        osb[:].rearrange("p (g d) -> p g d", d=D),
                    )

                return epi

            epilogue_pending.append(make_epilogue(j, m, opsum))

        for epi in epilogue_pending:
            epi()

    nc.compile()
    return nc


def _plan(valid_lens):
    """Sort heads by valid_len desc, deal round-robin across cores.

    Returns (assign [NCORES, SLOTS] head indices, m_list [SLOTS] chunk counts).
    """
    order = np.argsort(-valid_lens, kind="stable")
    assign = order.reshape(SLOTS, NCORES).T  # [core, slot]
    m_list = []
    for j in range(SLOTS):
        vmax = int(valid_lens[assign[:, j]].max())
        m_list.append(min(NCH, max(1, math.ceil(vmax / CHUNK))))
    return assign, m_list


def _run(queries, keys, values, valid_lens, trace=False):
    queries = np.ascontiguousarray(np.asarray(queries, dtype=np.float32))
    keys = np.ascontiguousarray(np.asarray(keys, dtype=np.float32))
    values = np.ascontiguousarray(np.asarray(values, dtype=np.float32))
    valid_lens = np.asarray(valid_lens, dtype=np.int32)

    assign, m_list = _plan(valid_lens)

    key = tuple(m_list)
    nc = _program_cache.get(key)
    if nc is None:
        nc = _build_program(m_list)
        _program_cache[key] = nc

    kk = np.arange(L, dtype=np.int64)
    in_maps = []
    for i in range(NCORES):
        heads = assign[i]
        mask = np.where(
            kk[None, :] < valid_lens[heads][:, None], 0.0, MASK_VALUE
        ).astype(np.float32)  # [SLOTS, L]
        # mb[p, j*NCH+c] = mask for key index c*128+p of slot j.
        mb = np.transpose(mask.reshape(SLOTS, NCH, CHUNK), (2, 0, 1)).reshape(
            CHUNK, SLOTS * NCH
        )
        in_maps.append(
            {
                "q": queries[heads],
                "k": keys[heads],
                "v": values[heads],
                "mb": np.ascontiguousarray(mb),
            }
        )

    res = run_bass_kernel_spmd(nc, in_maps, list(range(NCORES)), trace=trace)

    out = np.empty((BH, L, D), dtype=np.float32)
    for i in range(NCORES):
        out[assign[i]] = res.results[i]["o"]

    # valid_len == 0: reference softmaxes an all-masked row -> uniform weights.
    for h in np.nonzero(valid_lens == 0)[0]:
        out[h] = values[h].mean(axis=0, keepdims=True)

    return out, res


def kernel(queries, keys, values, valid_lens):
    out, _ = _run(queries, keys, values, valid_lens)
    return out


# revision 26
# speedup vs baseline: 1.0557x; 1.0557x over previous
"""Bass/Tile kernel for masked dot-product attention on 8 Trainium2 cores.

Problem: queries/keys/values [128, 1024, 64] fp32, valid_lens [128] int32.
  out[b] = softmax(mask(Q K^T / 8, valid_lens[b])) @ V

Strategy:
  * Shard the 128 batch*heads across 8 cores, 16 head-slots per core.
    Heads are sorted by valid_len (descending) and dealt round-robin so
    every core gets the same per-slot chunk count -> one SPMD program.
  * Per head, only ceil(valid_len/128) key chunks contribute (the rest are
    fully masked -> softmax weight exactly 0), so the program is
    specialized to skip them (~45% of the work for uniform valid_lens).
  * Layout: compute S^T = K Q^T chunkwise on the PE ([128 k x 1024 q]),
    so the PV matmul can consume P^T directly as the moving operand.
    Masking + 1/sqrt(d) scaling + exp run as a single ScalarE activation
    (bias = per-partition mask column of 0 / -1e6; no max subtraction is
    needed: scores are bounded and exp(-1e6) underflows to exactly 0,
    matching the fp32 reference).
  * Softmax denominators come free: a ones-column is appended to V, so
    the PV accumulation produces [O^T ; sum_k P^T] in one pass.
    Normalization happens after a final PE transpose, where the
    denominator is a per-partition scalar.
  * Heads with valid_len == 0 (reference: uniform attention) are fixed up
    on the host with the exact reference semantics (mean of V).
"""

import math
from contextlib import ExitStack

import numpy as np

import concourse.bass as bass  # noqa: F401  (engine namespaces live on the nc)
import concourse.mybir as mybir
import concourse.tile as tile
from concourse import bacc
from concourse.bass_utils import run_bass_kernel_spmd
from concourse.masks import make_identity

BH, L, D = 128, 1024, 64
NCORES = 8
SLOTS = BH // NCORES  # 16
CHUNK = 128
NCH = L // CHUNK  # 8
MASK_VALUE = -1000000.0
F32 = mybir.dt.float32
MM_DT = mybir.dt.float16  # 1 cyc/row on PE, ~2^-11 operand quantization

_program_cache: dict = {}


def _build_program(m_list):
    nc = bacc.Bacc("TRN2", target_bir_lowering=False, debug=False)
    q_d = nc.dram_tensor("q", [SLOTS, L, D], F32, kind="ExternalInput").ap()
    k_d = nc.dram_tensor("k", [SLOTS, L, D], F32, kind="ExternalInput").ap()
    v_d = nc.dram_tensor("v", [SLOTS, L, D], F32, kind="ExternalInput").ap()
    mb_d = nc.dram_tensor("mb", [CHUNK, SLOTS * NCH], F32, kind="ExternalInput").ap()
    scr = [
        nc.dram_tensor(f"scr{j}", [2 * L, 2 * D], MM_DT).ap() for j in range(SLOTS)
    ]
    o_d = nc.dram_tensor("o", [SLOTS, L, D], F32, kind="ExternalOutput").ap()

    Exp = mybir.ActivationFunctionType.Exp

    with tile.TileContext(nc) as tc, ExitStack() as ctx:
        const = ctx.enter_context(tc.tile_pool(name="const", bufs=1))
        ident = const.tile([128, 128], F32)
        make_identity(nc, ident)
        mb = const.tile([CHUNK, SLOTS * NCH], F32)
        nc.sync.dma_start(mb[:], mb_d[:])
        ones = const.tile([128, 1], F32)
        nc.gpsimd.memset(ones[:], 1.0)
        actwarm = const.tile([128, 1], F32, tag="actwarm")
        nc.scalar.activation(actwarm[:], ones[:], Exp, bias=0.0, scale=1.0)

        qpf_p = ctx.enter_context(tc.tile_pool(name="qpf", bufs=4))
        qpb_p = ctx.enter_context(tc.tile_pool(name="qpb", bufs=3))
        qt_p = ctx.enter_context(tc.tile_pool(name="qt", bufs=4))
        kt_p = ctx.enter_context(tc.tile_pool(name="kt", bufs=3))
        vnat_p = ctx.enter_context(tc.tile_pool(name="vnat", bufs=4))
        vp_p = ctx.enter_context(tc.tile_pool(name="vp", bufs=4))
        pt_p = ctx.enter_context(tc.tile_pool(name="pt", bufs=4))
        ot_p = ctx.enter_context(tc.tile_pool(name="ot", bufs=2))
        osb_p = ctx.enter_context(tc.tile_pool(name="osb", bufs=4))
        rec_p = ctx.enter_context(tc.tile_pool(name="rec", bufs=4))

        # PSUM: 8 banks. "s": S^T tiles + epilogue transposes (2 x 2 banks);
        # "ops": PV accumulators (2 x 2 banks).
        s_ps = ctx.enter_context(tc.tile_pool(name="s", bufs=2, space="PSUM"))
        o_ps = ctx.enter_context(tc.tile_pool(name="ops", bufs=4, space="PSUM"))

        # Dense matmul burst to flip the PE HAM clock-gate to full rate
        # (~3.4us of contiguous activity required) before real work starts.
        warm = const.tile([128, 512], MM_DT, tag="warm")
        nc.gpsimd.memset(warm[:], 0.5)
        wps = o_ps.tile([128, 512], F32, tag="ops")  # noqa
        for i in range(14):
            nc.tensor.matmul(
                wps[:], warm[:, 0:128], warm[:], start=True, stop=True
            )

        zt = const.tile([128, L], MM_DT, tag="zt")
        nc.gpsimd.memset(zt[:], 0.0)

        def load_head(j, m):
            """Q and K panels -> one fp16 DRAM panel -> XBAR-transposed SBUF.

            Returns qkt [128, L + m*CHUNK] fp16: cols 0:L are Q^T (rows 0-63
            real, 64-127 zero), cols L: are K^T chunks.
            """
            nrows = L + m * CHUNK
            nch = NCH + m
            pf = qpf_p.tile([128, L], F32, tag="pf", name=f"pf{j}")
            nc.sync.dma_start(
                pf[:, 0 : NCH * D].rearrange("p (c d) -> p c d", d=D),
                q_d[j].rearrange("(c p) d -> p c d", p=CHUNK),
            )
            nc.sync.dma_start(
                pf[:, NCH * D : NCH * D + m * D].rearrange("p (c d) -> p c d", d=D),
                k_d[j, 0 : m * CHUNK].rearrange("(c p) d -> p c d", p=CHUNK),
            )
            # Staging: data in cols c*128+0:64, zeros in c*128+64:128, so one
            # contiguous store fills the whole panel (incl. the zero half).
            pb = qpb_p.tile([128, 2 * L], MM_DT, tag="pb", name=f"pb{j}")
            pb3 = pb[:, 0 : nch * CHUNK].rearrange("p (c d) -> p c d", d=CHUNK)
            nc.gpsimd.memset(pb3[:, :, D : CHUNK], 0.0)
            nc.vector.tensor_copy(
                pb3[:, :, 0:D],
                pf[:, 0 : nch * D].rearrange("p (c d) -> p c d", d=D),
            )
            nc.sync.dma_start(
                scr[j][0:nrows].rearrange("(c p) d -> p c d", p=CHUNK),
                pb3,
            )
            qkt = qt_p.tile([128, 2 * L], MM_DT, tag="qt", name=f"qkt{j}")
            nc.sync.dma_start_transpose(qkt[:, 0:nrows], scr[j][0:nrows, :])

            # V chunks: [V_c | ones | zeros] -> M=128 stationary panels.
            vnat = vnat_p.tile([128, NCH * D], F32, tag="vnat", name=f"vn{j}")
            nc.gpsimd.dma_start(
                vnat[:, 0 : m * D].rearrange("p (c d) -> p c d", d=D),
                v_d[j, 0 : m * CHUNK].rearrange("(c p) d -> p c d", p=CHUNK),
            )
            vp = vp_p.tile([128, NCH * CHUNK], MM_DT, tag="vp", name=f"vp{j}")
            nc.vector.tensor_copy(
                vp[:].rearrange("p (c e) -> p c e", e=CHUNK)[:, 0:m, 0:D],
                vnat[:, 0 : m * D].rearrange("p (c d) -> p c d", d=D),
            )
            for c in range(m):
                base = c * CHUNK
                nc.vector.tensor_copy(vp[:, base + D : base + D + 1], ones[:])
                nc.vector.tensor_copy(
                    vp[:, base + D + 1 : base + CHUNK], zt[:, 0 : CHUNK - D - 1]
                )
            return qkt, vp

        epilogue_pending = []
        pending = [load_head(jj, m_list[jj]) for jj in range(min(3, SLOTS))]
        for j in range(SLOTS):
            m = m_list[j]
            qkt, vp = pending.pop(0)
            if j + 3 < SLOTS:
                pending.append(load_head(j + 3, m_list[j + 3]))

            opsum = [
                o_ps.tile([128, 512], F32, tag="ops", name=f"op{j}_{h}")
                for h in range(2)
            ]
            pts = {}

            def emit_pv(c):
                vl = vp[:, c * CHUNK : (c + 1) * CHUNK]
                for h in range(2):
                    nc.tensor.matmul(
                        opsum[h][:],
                        vl,
                        pts[c][:, h * 512 : (h + 1) * 512],
                        start=(c == 0),
                        stop=(c == m - 1),
                    )

            for c in range(m):
                s = s_ps.tile([128, L], F32, tag="s", name=f"s_{j}_{c}")
                for h in range(2):
                    nc.tensor.matmul(
                        s[:, h * 512 : (h + 1) * 512],
                        qkt[:, L + c * 128 : L + (c + 1) * 128],
                        qkt[:, h * 512 : (h + 1) * 512],
                        start=True,
                        stop=True,
                    )
                if c >= 1:
                    emit_pv(c - 1)
                if c == 1 and epilogue_pending:
                    epilogue_pending.pop(0)()
                pts[c] = pt_p.tile([128, L], MM_DT, tag="pt", name=f"pt{j}_{c}")
                col = j * NCH + c
                nc.scalar.activation(
                    pts[c][:], s[:], Exp, bias=mb[:, col : col + 1], scale=0.125
                )
            emit_pv(m - 1)

            def make_epilogue(j, m, opsum):
                def epi():
                    # Transpose [O^T ; denom] back (4 blocks per PSUM bank),
                    # normalize, one store.
                    ot = ot_p.tile([65, L], F32, tag="ot", name=f"ot{j}")
                    for h in range(2):
                        nc.vector.tensor_copy(
                            ot[:, h * 512 : (h + 1) * 512], opsum[h][0:65, :]
                        )
                    osb = osb_p.tile([128, NCH * D], F32, tag="osb", name=f"osb{j}")
                    for gg in range(2):
                        tt = o_ps.tile(
                            [128, 4 * 65], F32, tag="ops", name=f"tt{j}_{gg}"
                        )
                        for g4 in range(4):
                            g = 4 * gg + g4
                            nc.tensor.transpose(
                                tt[:, g4 * 65 : g4 * 65 + 65],
                                ot[:, g * 128 : (g + 1) * 128],
                                ident[0:65, 0:65],
                            )
                        rec = rec_p.tile(
                            [128, 4], F32, tag="rec", name=f"rec{j}_{gg}"
                        )
                        nc.vector.reciprocal(
                            rec[:],
                            tt[:].rearrange("p (g e) -> p g e", e=65)[:, :, 64],
                        )
                        for g4 in range(4):
                            g = 4 * gg + g4
                            nc.vector.tensor_scalar_mul(
                                osb[:, g * D : (g + 1) * D],
                                tt[:, g4 * 65 : g4 * 65 + 64],
                                rec[:, g4 : g4 + 1],
                            )
                    nc.gpsimd.dma_start(
                        o_d[j].rearrange("(g p) d -> p g d", p=CHUNK),
                        osb[:].rearrange("p (g d) -> p g d", d=D),
                    )

                return epi

            epilogue_pending.append(make_epilogue(j, m, opsum))

        for epi in epilogue_pending:
            epi()

    nc.compile()
    return nc


def _plan(valid_lens):
    """Sort heads by valid_len desc, deal round-robin across cores.

    Returns (assign [NCORES, SLOTS] head indices, m_list [SLOTS] chunk counts).
    """
    order = np.argsort(-valid_lens, kind="stable")
    assign = order.reshape(SLOTS, NCORES).T  # [core, slot]
    m_list = []
    for j in range(SLOTS):
        vmax = int(valid_lens[assign[:, j]].max())
        m_list.append(min(NCH, max(1, math.ceil(vmax / CHUNK))))
    return assign, m_list


def _run(queries, keys, values, valid_lens, trace=False):
    queries = np.ascontiguousarray(np.asarray(queries, dtype=np.float32))
    keys = np.ascontiguousarray(np.asarray(keys, dtype=np.float32))
    values = np.ascontiguousarray(np.asarray(values, dtype=np.float32))
    valid_lens = np.asarray(valid_lens, dtype=np.int32)

    assign, m_list = _plan(valid_lens)

    key = tuple(m_list)
    nc = _program_cache.get(key)
    if nc is None:
        nc = _build_program(m_list)
        _program_cache[key] = nc

    kk = np.arange(L, dtype=np.int64)
    in_maps = []
    for i in range(NCORES):
        heads = assign[i]
        mask = np.where(
            kk[None, :] < valid_lens[heads][:, None], 0.0, MASK_VALUE
        ).astype(np.float32)  # [SLOTS, L]
        # mb[p, j*NCH+c] = mask for key index c*128+p of slot j.
        mb = np.transpose(mask.reshape(SLOTS, NCH, CHUNK), (2, 0, 1)).reshape(
            CHUNK, SLOTS * NCH
        )
        in_maps.append(
            {
                "q": queries[heads],
                "k": keys[heads],
                "v": values[heads],
                "mb": np.ascontiguousarray(mb),
            }
        )

    res = run_bass_kernel_spmd(nc, in_maps, list(range(NCORES)), trace=trace)

    out = np.empty((BH, L, D), dtype=np.float32)
    for i in range(NCORES):
        out[assign[i]] = res.results[i]["o"]

    # valid_len == 0: reference softmaxes an all-masked row -> uniform weights.
    for h in np.nonzero(valid_lens == 0)[0]:
        out[h] = values[h].mean(axis=0, keepdims=True)

    return out, res


def kernel(queries, keys, values, valid_lens):
    out, _ = _run(queries, keys, values, valid_lens)
    return out
